# revision 8
# baseline (speedup 1.0000x reference)
"""Trainium2 Bass kernel: batched 3x3 polar decomposition + tangent projection.

reference semantics (per matrix n of N=2,000,000):
    u, _, vT = svd(x);  xm = u @ vT          (polar factor)
    vt = 0.5*(v - xm @ v^T @ xm)

Implementation: determinant-scaled Newton iteration for the polar factor
(gamma-form, scale-invariant):  X <- X + sign(d)|d|^(-1/3) * cof(X)
with cof() the signed cofactor matrix (X^{-T} = cof(X)/det(X)); final
iteration applies exact alpha*X + beta*cof(X) with an extra 1/sqrt(2)
folded in so the projection needs no 0.5 on the quadratic term:
    vt = 0.5 v - xmh (xmh^T v)^T,   xmh = xm/sqrt(2).

Data layout: SoA "planes" [128, 3, 3, F] per tile; the cyclic cofactor
index patterns are expressed with negative-stride access patterns
(rows (2,0) = start 2, step -2), split into 2x2 blocks per product.

Sharding: batch split evenly across 8 NeuronCores, zero communication.
"""

import numpy as np

import concourse.bass as bass
import concourse.bacc as bacc
import concourse.mybir as mybir
import concourse.tile as tile
from concourse.bass_utils import run_bass_kernel_spmd

dt = mybir.dt.float32
AF = mybir.ActivationFunctionType
OP = mybir.AluOpType

NCORES = 8
N_TOTAL = 2_000_000
N_CORE = N_TOTAL // NCORES      # 250_000

# device tiling (full config)
F = 652                          # free-dim elements per partition per tile
TILES = 3
ITERS = 6                        # total Newton iterations (incl. final)

LN2 = float(np.log(2.0))
DELTA = 1e-15                    # det bump (unsticks exact-zero fp32 det)
EPS = 1e-35                      # clamp inside Ln


def _r20(ap4, dim):
    """Rows/cols (2,0): start 2, step -2, count 2 along given middle dim."""
    if dim == 1:
        return ap4[:, 2::-2, :, :]
    return ap4[:, :, 2::-2, :]


def _cof_blocks(nc, Xv, Ca, Cb, f):
    """Emit the 8 product instructions + combine for the signed cofactor.

    C[u,v] = X[(u+1)%3,(v+1)%3]*X[(u+2)%3,(v+2)%3]
           - X[(u+1)%3,(v+2)%3]*X[(u+2)%3,(v+1)%3]
    u-blocks {0,1}|{2}: r1=(1,2)|(0), r2=(2,0)|(1)
    v-blocks {0,1}|{2}: c1=(1,2)|(0), c2=(2,0)|(1)
    Writes P1 products into Ca, P2 into Cb, then Ca -= Cb... actually
    Ca = P1, Cb = P1b; final C = Ca - Cb done by caller.
    """
    X = Xv
    r12 = lambda a: a[:, 1:3, :, :]
    r20 = lambda a: a[:, 2::-2, :, :]
    r0 = lambda a: a[:, 0:1, :, :]
    r1 = lambda a: a[:, 1:2, :, :]
    c12 = lambda a: a[:, :, 1:3, :]
    c20 = lambda a: a[:, :, 2::-2, :]
    c0 = lambda a: a[:, :, 0:1, :]
    c1 = lambda a: a[:, :, 1:2, :]

    # Ta = X[r1,c1] * X[r2,c2]
    nc.vector.tensor_mul(Ca[:, 0:2, 0:2, :], c12(r12(X)), c20(r20(X)))
    nc.vector.tensor_mul(Ca[:, 0:2, 2:3, :], c0(r12(X)), c1(r20(X)))
    nc.vector.tensor_mul(Ca[:, 2:3, 0:2, :], c12(r0(X)), c20(r1(X)))
    nc.vector.tensor_mul(Ca[:, 2:3, 2:3, :], c0(r0(X)), c1(r1(X)))
    # Tb = X[r1,c2] * X[r2,c1]
    nc.vector.tensor_mul(Cb[:, 0:2, 0:2, :], c20(r12(X)), c12(r20(X)))
    nc.vector.tensor_mul(Cb[:, 0:2, 2:3, :], c1(r12(X)), c0(r20(X)))
    nc.vector.tensor_mul(Cb[:, 2:3, 0:2, :], c20(r0(X)), c12(r1(X)))
    nc.vector.tensor_mul(Cb[:, 2:3, 2:3, :], c1(r0(X)), c0(r1(X)))


def build_nc(f=F, tiles=TILES, iters=ITERS):
    """Per-core Bass graph. Inputs x, v: [9, tiles*128*f] f32 planes (plane
    p = 3*i+j holds entry (i,j) of each matrix, matrix m at column m);
    output "out" same layout holding vt."""
    npt = 128 * f
    np_tot = npt * tiles

    nc = bacc.Bacc()
    x = nc.declare_dram_parameter("x", [9, np_tot], dt, isOutput=False)
    v = nc.declare_dram_parameter("v", [9, np_tot], dt, isOutput=False)
    out = nc.declare_dram_parameter("out", [9, np_tot], dt, isOutput=True)

    with tile.TileContext(nc) as tc:
        with tc.tile_pool(name="p", bufs=1) as pool:
            c_eps = pool.tile([128, 1], dt, tag="c_eps")
            c_b2 = pool.tile([128, 1], dt, tag="c_b2")
            nc.vector.memset(c_eps[:, :], EPS)
            nc.vector.memset(c_b2[:, :], -1.5 * LN2)
            for t in range(tiles):
                sl = slice(t * npt, (t + 1) * npt)

                X = pool.tile([128, 9, f], dt, tag="X", bufs=2)
                vb = pool.tile([128, 9, f], dt, tag="vb", bufs=2)
                nc.sync.dma_start(X[:, :, :], x[:, sl].rearrange("p (q e) -> q p e", q=128))
                nc.sync.dma_start(vb[:, :, :], v[:, sl].rearrange("p (q e) -> q p e", q=128))
                X4 = X.rearrange("q (a b) e -> q a b e", a=3)
                vb4 = vb.rearrange("q (a b) e -> q a b e", a=3)

                C = pool.tile([128, 3, 3, f], dt, tag="C")
                Tb = pool.tile([128, 3, 3, f], dt, tag="Tb")

                for it in range(iters):
                    last = it == iters - 1
                    D = pool.tile([128, 3, f], dt, tag="D")
                    tq = pool.tile([128, f], dt, tag="tq")
                    ds = pool.tile([128, f], dt, tag="ds")
                    d2 = pool.tile([128, f], dt, tag="d2")
                    L = pool.tile([128, f], dt, tag="L")
                    w = pool.tile([128, f], dt, tag="w")
                    ga = pool.tile([128, f], dt, tag="ga")

                    _cof_blocks(nc, X4, C, Tb, f)
                    nc.vector.tensor_sub(C[:, :, :, :], C[:, :, :, :], Tb[:, :, :, :])

                    # det = sum_j X[0,j]*C[0,j]  (+ DELTA bump, fused)
                    nc.vector.tensor_mul(D[:, :, :], X4[:, 0, :, :], C[:, 0, :, :])
                    nc.vector.tensor_add(tq[:, :], D[:, 0, :], D[:, 1, :])
                    nc.vector.scalar_tensor_tensor(
                        ds[:, :], tq[:, :], DELTA, D[:, 2, :], OP.add, OP.add
                    )
                    nc.scalar.activation(d2[:, :], ds[:, :], AF.Square)
                    nc.scalar.activation(L[:, :], d2[:, :], AF.Ln, bias=c_eps[:, :])

                    if not last:
                        # gamma = ds * exp(-2/3 * L)
                        nc.scalar.activation(w[:, :], L[:, :], AF.Exp, scale=-2.0 / 3.0)
                        nc.vector.tensor_mul(ga[:, :], ds[:, :], w[:, :])
                        gb = ga.unsqueeze(1).unsqueeze(1).broadcast_to((128, 3, 3, f))
                        nc.vector.tensor_mul(Tb[:, :, :, :], C[:, :, :, :], gb)
                        nc.vector.tensor_add(X4, X4, Tb[:, :, :, :])
                    else:
                        # xmh = xm/sqrt2 = alpha*X + beta*C
                        # alpha = exp(-L/6 - 1.5 ln2); beta = ds*exp(-5L/6 - 1.5 ln2)
                        al = pool.tile([128, f], dt, tag="al")
                        be = pool.tile([128, f], dt, tag="be")
                        nc.scalar.activation(al[:, :], L[:, :], AF.Exp, scale=-1.0 / 6.0, bias=c_b2[:, :])
                        nc.scalar.activation(w[:, :], L[:, :], AF.Exp, scale=-5.0 / 6.0, bias=c_b2[:, :])
                        nc.vector.tensor_mul(be[:, :], ds[:, :], w[:, :])
                        ab = al.unsqueeze(1).unsqueeze(1).broadcast_to((128, 3, 3, f))
                        bb = be.unsqueeze(1).unsqueeze(1).broadcast_to((128, 3, 3, f))
                        nc.vector.tensor_mul(Tb[:, :, :, :], X4, ab)
                        nc.vector.tensor_mul(C[:, :, :, :], C[:, :, :, :], bb)
                        nc.vector.tensor_add(C[:, :, :, :], Tb[:, :, :, :], C[:, :, :, :])
                        # C now holds xmh

                # ---- tangent projection: vt = 0.5 v - xmh (xmh^T v)^T ----
                Wf = pool.tile([128, 3, 3, f], dt, tag="Wf")
                for k in range(3):
                    # Wf[k,j] = sum_i xmh[i,k]*v[i,j]
                    ck = C[:, 0:3, k : k + 1, :].broadcast_to((128, 3, 3, f))
                    nc.vector.tensor_mul(Tb[:, :, :, :], ck, vb4)
                    nc.vector.tensor_add(Wf[:, k, :, :], Tb[:, 0, :, :], Tb[:, 1, :, :])
                    nc.vector.tensor_add(Wf[:, k, :, :], Wf[:, k, :, :], Tb[:, 2, :, :])
                for k in range(3):
                    # P[i,j] = xmh[i,k]*Wf[j,k];  out = 0.5 v - sum_k P
                    cki = C[:, 0:3, k : k + 1, :].broadcast_to((128, 3, 3, f))
                    wkb = Wf[:, 0:3, k, :].unsqueeze(1).broadcast_to((128, 3, 3, f))
                    nc.vector.tensor_mul(Tb[:, :, :, :], cki, wkb)
                    if k == 0:
                        nc.vector.scalar_tensor_tensor(
                            vb4, vb4, 0.5, Tb[:, :, :, :], OP.mult, OP.subtract
                        )
                    else:
                        nc.vector.tensor_sub(vb4, vb4, Tb[:, :, :, :])

                # ---- DMA out ----
                nc.sync.dma_start(out[:, sl].rearrange("p (q e) -> q p e", q=128), vb[:, :, :])

    nc.finalize()
    return nc


# ---------------- host side ----------------

def _to_planes(a, n_pad, fill_identity):
    """[N,3,3] f32 -> [9, n_pad] planes (plane 3i+j = entry (i,j))."""
    n = a.shape[0]
    flat = np.empty((9, n_pad), dtype=np.float32)
    flat[:, :n] = a.reshape(n, 9).T
    if n_pad > n:
        pad = np.zeros(9, dtype=np.float32)
        if fill_identity:
            pad[[0, 4, 8]] = 1.0
        flat[:, n:] = pad[:, None]
    return np.ascontiguousarray(flat)


_NC_CACHE = {}
LAST_RESULT = None


def _get_nc():
    key = (F, TILES, ITERS)
    if key not in _NC_CACHE:
        _NC_CACHE[key] = build_nc()
    return _NC_CACHE[key]


def kernel(x, v):
    x = np.asarray(x, dtype=np.float32)
    v = np.asarray(v, dtype=np.float32)
    n = x.shape[0]
    assert n == N_TOTAL, f"expected {N_TOTAL} matrices, got {n}"

    np_tot = 128 * F * TILES
    nc = _get_nc()

    in_maps = []
    for c in range(NCORES):
        sl = slice(c * N_CORE, (c + 1) * N_CORE)
        in_maps.append(
            {
                "x": _to_planes(x[sl], np_tot, fill_identity=True),
                "v": _to_planes(v[sl], np_tot, fill_identity=False),
            }
        )

    global LAST_RESULT
    res = run_bass_kernel_spmd(nc, in_maps, core_ids=list(range(NCORES)))
    LAST_RESULT = res

    outp = np.empty((n, 3, 3), dtype=np.float32)
    for c in range(NCORES):
        o = res.results[c]["out"]  # [9, np_tot]
        outp[c * N_CORE : (c + 1) * N_CORE] = (
            o[:, :N_CORE].T.reshape(N_CORE, 3, 3)
        )
    return outp
